# revision 7
# baseline (speedup 1.0000x reference)
"""Trainium2 Bass kernel for nn_ErdosLoss (graph loss function).

Math (reference reformulated, validated to ~1e-6 rel err):
  penalty:  log_score = scatter_add(log(1 - p + 1e-6), tgt)   over N nodes
            loss2 = mean(exp(log_score)) * 9600
  loss3:    p @ triu(H H^T, 1) @ p^T  ==  (||s||^2 - sum_e d_e p_e^2) / 2
            where s = scatter_add(p, tgt) + scatter_add(p * (1-m), src),
            m_e = (src_e == tgt_e)  (H rows are node *sets*: self-loops get a
            single 1), d_e = 2 - m_e.
  out = loss2 + 200 * loss3 / num_graphs,  num_graphs = max(batch) + 1.

Device strategy (8 NeuronCores, SPMD, two launches, no collectives):
  Launch skew across the 8 PJRT devices is ~30us here, so any cross-core
  barrier (AllReduce) inflates every core's exec time by the skew plus a
  ~13us collective.  Instead:
  - Phase 1 (8 cores, edge-sharded 750/core): scatter-add via one-hot
    matmul with node = 128*hi + lo decomposition (N padded to 4096).
    One-hots are built as a handful of *wide* DVE ops using stride-0
    broadcast APs (per-instruction overhead dominates at this size), cast
    to f16 (exact for 0/1 and for values' 10-bit precision needs), and
    contracted on TensorE into PSUM [128lo, 64] (= log_score | s).
    Each core writes a [128, 65] partial (log_score | s | dp2 rowsum).
  - Host gathers the 8 partials (pure data movement) into [128, 520].
  - Phase 2 (1 core): tree-add the 8 partials, exp/square row-sums,
    ones-matmul partition reduce, max(batch)+1 on device, final scalar.
"""

import os
import numpy as np

import concourse.bass as bass
import concourse.bacc as bacc
import concourse.mybir as mybir
import concourse.tile as tile
import concourse.bass_isa as bass_isa
from concourse import bass_utils

F32 = mybir.dt.float32
F16 = mybir.dt.float16
ALU = mybir.AluOpType
ACT = mybir.ActivationFunctionType
AX = mybir.AxisListType

N_NODES = 4000
N_EDGES = 6000
N_CORES = 8
N_PAD = 4096          # 128 * 32
HI = 32               # node hi-digits
LO = 128              # node lo-digits
PENALTY_SCALE = 16 * 200 * 3   # 9600
PAD_NODES = N_PAD - N_NODES    # 96 padded nodes, each contributes exp(0)=1

EPC = N_EDGES // N_CORES       # 750 edges per core
TPC = (EPC + 127) // 128       # 6 edge tiles per core

# edata column layout: 7 fields x T columns (lo pair and hi pair adjacent)
_F_TLO, _F_ULO, _F_THI, _F_UHI, _F_TF, _F_UF, _F_P = range(7)


def _common_inputs(nc):
    iota128 = nc.dram_tensor("iota128", [128, LO], F32, kind="ExternalInput").ap()
    iota32 = nc.dram_tensor("iota32", [128, HI], F32, kind="ExternalInput").ap()
    return iota128, iota32


def _build_phase1(T: int):
    """Per-core partial computation: out 'partial' [128, 65] f16."""
    nc = bacc.Bacc("TRN2", target_bir_lowering=False, debug=False, num_devices=1)

    # blob_a: what the one-hot builds need first; blob_b: values
    # blob_a: [iota128(128) | iota32(32) | tlo,ulo,thi,uhi (4T)]
    # blob_b: [cbias(2) | tf,uf,p (3T)]
    AW = LO + HI + 4 * T
    BW = 2 + 3 * T
    bloba = nc.dram_tensor("bloba", [128, AW], F32, kind="ExternalInput").ap()
    blobb = nc.dram_tensor("blobb", [128, BW], F32, kind="ExternalInput").ap()
    partiald = nc.dram_tensor("partial", [128, 65], F16, kind="ExternalOutput").ap()

    with tile.TileContext(nc) as tc:
        with (
            tc.tile_pool(name="const", bufs=1) as cpool,
            tc.tile_pool(name="work", bufs=1) as wpool,
            tc.tile_pool(name="psum", bufs=1, space="PSUM") as ppool,
        ):
            # warm the Ln ACT table while the input DMA is in flight
            wz = cpool.tile([128, 1], F32, tag="wz")
            nc.vector.memset(wz[:], 0.5)
            wb = cpool.tile([128, 1], F32, tag="wb")
            nc.gpsimd.memset(wb[:], 0.0)
            wo = cpool.tile([128, 1], F32, tag="wo")
            nc.scalar.activation(wo[:], wz[:], ACT.Ln, bias=wb[:])

            ba = cpool.tile([128, AW], F32, tag="ba")
            nc.sync.dma_start(ba[:], bloba)
            bb = cpool.tile([128, BW], F32, tag="bb")
            nc.sync.dma_start(bb[:], blobb)

            io128 = ba[:, 0:LO]
            io32 = ba[:, LO:LO + HI]
            lo_pair = ba[:, LO + HI:LO + HI + 2 * T]
            hi_pair = ba[:, LO + HI + 2 * T:LO + HI + 4 * T]
            cb = bb[:, 0:2]
            tf = bb[:, 2:2 + T]
            uf = bb[:, 2 + T:2 + 2 * T]
            pp = bb[:, 2 + 2 * T:2 + 3 * T]

            C = wpool.tile([128, 65], F16, tag="C")

            # ---- one-hots (f16, exact), few wide DVE ops via stride-0 APs
            H_all = wpool.tile([128, 2 * T * HI], F16, tag="H_all")
            nc.vector.tensor_tensor(
                H_all[:].rearrange("p (t h) -> p t h", h=HI),
                io32.rearrange("p (o h) -> p o h", o=1).to_broadcast((128, 2 * T, HI)),
                hi_pair.rearrange("p (t o) -> p t o", o=1).to_broadcast((128, 2 * T, HI)),
                op=ALU.is_equal,
            )
            A_all = wpool.tile([128, 2 * T * LO], F16, tag="A_all")
            nc.vector.tensor_tensor(
                A_all[:].rearrange("p (t l) -> p t l", l=LO),
                io128.rearrange("p (o l) -> p o l", o=1).to_broadcast((128, 2 * T, LO)),
                lo_pair.rearrange("p (t o) -> p t o", o=1).to_broadcast((128, 2 * T, LO)),
                op=ALU.is_equal,
            )
            # V = [logmsg | p] on the ACT engine (parallel to the DVE ops)
            V = wpool.tile([128, 2 * T], F32, tag="V")
            nc.scalar.activation(V[:, 0:T], pp, ACT.Ln, scale=-1.0, bias=cb[:, 1:2])
            nc.scalar.copy(V[:, T:2 * T], pp)
            # small per-edge prep on GpSimd (parallel to the DVE ops)
            m = wpool.tile([128, T], F32, tag="m")
            nc.vector.tensor_tensor(m[:], tf, uf, op=ALU.is_equal)
            valu = wpool.tile([128, T], F32, tag="valu")   # p * (1 - m)
            nc.vector.scalar_tensor_tensor(
                valu[:], m[:], 0.5, pp, op0=ALU.is_lt, op1=ALU.mult
            )
            # dp2 = p^2 (2 - m) = (valu + p) * p, row-summed
            tsum = wpool.tile([128, T], F32, tag="tsum")
            nc.vector.tensor_tensor(tsum[:], valu[:], pp, op=ALU.add)
            dp2scr = wpool.tile([128, T], F32, tag="dp2scr")
            dp2r = wpool.tile([128, 1], F32, tag="dp2r")
            nc.vector.scalar_tensor_tensor(
                dp2scr[:], tsum[:], 1.0, pp,
                op0=ALU.mult, op1=ALU.mult, accum_out=dp2r[:],
            )

            # RS_all: per tile i the contiguous [rp_i(32) | rst_i(32)]
            RS_all = wpool.tile([128, T * 64], F16, tag="RS_all")
            nc.vector.tensor_tensor(
                RS_all[:].rearrange("p (t o h) -> p o t h", o=2, h=HI),
                H_all[:, 0:T * HI].rearrange("p (o t h) -> p o t h", o=1, h=HI)
                    .to_broadcast((128, 2, T, HI)),
                V[:].rearrange("p (o t) -> p o t", o=2)
                    .rearrange("p o (t h) -> p o t h", h=1)
                    .to_broadcast((128, 2, T, HI)),
                op=ALU.mult,
            )
            rsu_all = wpool.tile([128, T * HI], F16, tag="rsu_all")
            nc.vector.tensor_tensor(
                rsu_all[:].rearrange("p (t h) -> p t h", h=HI),
                H_all[:, T * HI:2 * T * HI].rearrange("p (t h) -> p t h", h=HI),
                valu[:].rearrange("p (t o) -> p t o", o=1).to_broadcast((128, T, HI)),
                op=ALU.mult,
            )

            # ---- scatter-add matmuls: P12 = [log_score(32) | s(32)]
            P12 = ppool.tile([128, 64], F32, tag="P12")
            for i in range(T):
                nc.tensor.matmul(
                    P12[:, 0:64],
                    A_all[:, i * LO:(i + 1) * LO],
                    RS_all[:, i * 64:(i + 1) * 64],
                    start=(i == 0), stop=False, skip_group_check=True,
                )
            for i in range(T):
                nc.tensor.matmul(
                    P12[:, 32:64],
                    A_all[:, (T + i) * LO:(T + i + 1) * LO],
                    rsu_all[:, i * HI:(i + 1) * HI],
                    start=False, stop=(i == T - 1), skip_group_check=True,
                )

            nc.scalar.copy(C[:, 0:64], P12[:])
            nc.scalar.copy(C[:, 64:65], dp2r[:])
            nc.sync.dma_start(partiald, C[:])

    nc.compile()
    return nc


def _build_phase2():
    """Combine 8 partials -> final scalar. Runs on one core."""
    nc = bacc.Bacc("TRN2", target_bir_lowering=False, debug=False, num_devices=1)

    partsd = nc.dram_tensor("parts", [128, 8 * 65], F16, kind="ExternalInput").ap()
    # misc: [ones(1) | cbias(2) | batchf(32)]
    miscd = nc.dram_tensor("misc", [128, 35], F32, kind="ExternalInput").ap()
    outd = nc.dram_tensor("out", [1, 1], F32, kind="ExternalOutput").ap()

    with tile.TileContext(nc) as tc:
        with (
            tc.tile_pool(name="pool", bufs=1) as pool,
            tc.tile_pool(name="psum", bufs=1, space="PSUM") as ppool,
        ):
            wz = pool.tile([128, 1], F32, tag="wz")
            nc.vector.memset(wz[:], 0.5)
            wb = pool.tile([128, 1], F32, tag="wb")
            nc.gpsimd.memset(wb[:], 0.0)
            wo = pool.tile([128, 1], F32, tag="wo")
            nc.scalar.activation(wo[:], wz[:], ACT.Exp, bias=wb[:])

            mi = pool.tile([128, 35], F32, tag="mi")
            nc.sync.dma_start(mi[:], miscd)
            pt = pool.tile([128, 8 * 65], F16, tag="pt")
            nc.sync.dma_start(pt[:], partsd)
            ones_t = mi[:, 0:1]
            cb = mi[:, 1:3]
            bt = mi[:, 3:35]
            bzero = cb[:, 0:1]

            # num_graphs = max(batch) + 1 on device, off the critical path
            bmax = pool.tile([128, 1], F32, tag="bmax")
            nc.vector.tensor_reduce(bmax[:], bt, axis=AX.X, op=ALU.max)
            ball = pool.tile([128, 1], F32, tag="ball")
            nc.gpsimd.partition_all_reduce(
                ball[:], bmax[:], channels=128, reduce_op=bass_isa.ReduceOp.max
            )
            ng = pool.tile([1, 1], F32, tag="ng")
            nc.vector.tensor_scalar_add(ng[:], ball[0:1, 0:1], 1.0)
            rng = pool.tile([1, 1], F32, tag="rng")
            nc.vector.reciprocal(rng[:], ng[:])

            # 8-way partial sum in one strided reduce: C2[p,x] = sum_c pt[p,c*65+x]
            C2 = pool.tile([128, 65], F32, tag="C2")
            nc.vector.tensor_reduce(
                C2[:], pt[:].rearrange("p (c x) -> p x c", c=8), axis=AX.X, op=ALU.add
            )

            R = pool.tile([128, 3], F32, tag="R")
            scr1 = pool.tile([128, HI], F32, tag="scr1")
            nc.scalar.activation(scr1[:], C2[:, 0:32], ACT.Exp, bias=bzero,
                                 accum_out=R[:, 0:1])
            scr2 = pool.tile([128, HI], F32, tag="scr2")
            nc.vector.scalar_tensor_tensor(
                scr2[:], C2[:, 32:64], 1.0, C2[:, 32:64],
                op0=ALU.mult, op1=ALU.mult, accum_out=R[:, 1:2],
            )
            nc.vector.tensor_copy(R[:, 2:3], C2[:, 64:65])

            F = ppool.tile([1, 3], F32, tag="F")
            nc.tensor.matmul(F[:], ones_t, R[:], start=True, stop=True)
            Fs = pool.tile([1, 3], F32, tag="Fs")
            nc.scalar.copy(Fs[:], F[:])

            l2 = pool.tile([1, 1], F32, tag="l2")
            nc.vector.tensor_scalar(
                l2[:], Fs[:, 0:1], -float(PAD_NODES), PENALTY_SCALE / N_NODES,
                op0=ALU.add, op1=ALU.mult,
            )
            d32 = pool.tile([1, 1], F32, tag="d32")
            nc.vector.tensor_tensor(d32[:], Fs[:, 1:2], Fs[:, 2:3], op=ALU.subtract)
            t2s = pool.tile([1, 1], F32, tag="t2s")
            nc.vector.scalar_tensor_tensor(
                t2s[:], d32[:], 100.0, rng[:], op0=ALU.mult, op1=ALU.mult
            )
            res = pool.tile([1, 1], F32, tag="res")
            nc.vector.tensor_tensor(res[:], l2[:], t2s[:], op=ALU.add)
            nc.sync.dma_start(outd, res[:])

    nc.compile()
    return nc


def _pack_core(tt, uu, p, T):
    """Pack one core's edge shard into the [128, 7*T] fp32 edata layout."""
    ne = tt.shape[0]
    npad = T * 128

    def pad(a, fill):
        out = np.full(npad, fill, np.float64)
        out[:ne] = a
        return out.reshape(T, 128).T.astype(np.float32)  # [128, T]

    t_lo = pad(tt % 128, 0.0)
    t_hi = pad(tt // 128, float(HI))     # sentinel hi -> matches nothing
    u_lo = pad(uu % 128, 0.0)
    u_hi = pad(uu // 128, float(HI))
    tf = pad(tt, 0.0)
    uf = pad(uu, 0.0)                    # pad: tf==uf -> m=1, but p=0
    pf = pad(p, 0.0)
    return np.concatenate([t_lo, u_lo, t_hi, u_hi, tf, uf, pf], axis=1)


_CACHE = {}


def _get(name, builder, *a):
    if name not in _CACHE:
        _CACHE[name] = builder(*a)
    return _CACHE[name]


def kernel(x, edge_index, edge_feature, batch, _trace=False):
    x = np.asarray(x)
    ei = np.asarray(edge_index).astype(np.int64)
    p = np.asarray(edge_feature).astype(np.float32)[:, 0]
    batch = np.asarray(batch).astype(np.int64)

    uu_all = ei[0].astype(np.float64)
    tt_all = ei[1].astype(np.float64)

    iota128 = np.tile(np.arange(LO, dtype=np.float32), (128, 1))
    iota32 = np.tile(np.arange(HI, dtype=np.float32), (128, 1))
    ones = np.ones((128, 1), np.float32)
    cbias = np.zeros((128, 2), np.float32)
    cbias[:, 1] = 1.0 + 1e-6
    bpad = np.zeros(N_PAD, np.float32)
    bpad[:N_NODES] = batch.astype(np.float32)
    batchf = bpad.reshape(128, HI)

    # ---- phase 1: per-core partials (no cross-core dependencies)
    nc1 = _get("p1", _build_phase1, TPC)
    in_maps = []
    for c in range(N_CORES):
        sl = slice(c * EPC, (c + 1) * EPC)
        ed = _pack_core(tt_all[sl], uu_all[sl], p[sl], TPC)
        T = TPC
        bloba = np.concatenate([iota128, iota32, ed[:, 0:4 * T]], axis=1)
        blobb = np.concatenate([cbias, ed[:, 4 * T:7 * T]], axis=1)
        in_maps.append({"bloba": bloba, "blobb": blobb})
    r1 = bass_utils.run_bass_kernel_spmd(
        nc1, in_maps, core_ids=list(range(N_CORES)), trace=_trace
    )

    # gather/unshard the per-core partials (pure data movement)
    parts = np.concatenate(
        [np.asarray(r1.results[c]["partial"]) for c in range(N_CORES)], axis=1
    ).astype(np.float16)

    # ---- phase 2: combine on one core
    nc2 = _get("p2", _build_phase2)
    misc = np.concatenate([ones, cbias, batchf], axis=1)
    r2 = bass_utils.run_bass_kernel_spmd(
        nc2, [{"parts": parts, "misc": misc}], core_ids=[0], trace=_trace,
    )
    out = np.asarray(r2.results[0]["out"], dtype=np.float32).reshape(1, 1)
    if _trace:
        kernel.last_results = (r1, r2)
    return out


# revision 8
# speedup vs baseline: 1.1500x; 1.1500x over previous
"""Trainium2 Bass kernel for nn_ErdosLoss (graph loss function).

Math (reference reformulated, validated to ~1e-6 rel err):
  penalty:  log_score = scatter_add(log(1 - p + 1e-6), tgt)   over N nodes
            loss2 = mean(exp(log_score)) * 9600
  loss3:    p @ triu(H H^T, 1) @ p^T  ==  (||s||^2 - sum_e d_e p_e^2) / 2
            where s = scatter_add(p, tgt) + scatter_add(p * (1-m), src),
            m_e = (src_e == tgt_e)  (H rows are node *sets*: self-loops get a
            single 1), d_e = 2 - m_e.
  out = loss2 + 200 * loss3 / num_graphs,  num_graphs = max(batch) + 1.

Device strategy (8 NeuronCores, SPMD, two launches, no collectives):
  Launch skew across the 8 PJRT devices is ~30us here, so any cross-core
  barrier (AllReduce) inflates every core's exec time by the skew plus a
  ~13us collective.  Instead:
  - Phase 1 (8 cores, edge-sharded 750/core): scatter-add via one-hot
    matmul with node = 128*hi + lo decomposition (N padded to 4096).
    One-hots are built as a handful of *wide* DVE ops using stride-0
    broadcast APs (per-instruction overhead dominates at this size), cast
    to f16 (exact for 0/1 and for values' 10-bit precision needs), and
    contracted on TensorE into PSUM [128lo, 64] (= log_score | s).
    Each core writes a [128, 65] partial (log_score | s | dp2 rowsum).
  - Host gathers the 8 partials (pure data movement) into [128, 520].
  - Phase 2 (1 core): tree-add the 8 partials, exp/square row-sums,
    ones-matmul partition reduce, max(batch)+1 on device, final scalar.
"""

import os
import numpy as np

import concourse.bass as bass
import concourse.bacc as bacc
import concourse.mybir as mybir
import concourse.tile as tile
import concourse.bass_isa as bass_isa
from concourse import bass_utils

F32 = mybir.dt.float32
F16 = mybir.dt.float16
ALU = mybir.AluOpType
ACT = mybir.ActivationFunctionType
AX = mybir.AxisListType

N_NODES = 4000
N_EDGES = 6000
N_CORES = 8
N_PAD = 4096          # 128 * 32
HI = 32               # node hi-digits
LO = 128              # node lo-digits
PENALTY_SCALE = 16 * 200 * 3   # 9600
PAD_NODES = N_PAD - N_NODES    # 96 padded nodes, each contributes exp(0)=1

EPC = N_EDGES // N_CORES       # 750 edges per core
TPC = (EPC + 127) // 128       # 6 edge tiles per core

# edata column layout: 7 fields x T columns (lo pair and hi pair adjacent)
_F_TLO, _F_ULO, _F_THI, _F_UHI, _F_TF, _F_UF, _F_P = range(7)


def _common_inputs(nc):
    iota128 = nc.dram_tensor("iota128", [128, LO], F32, kind="ExternalInput").ap()
    iota32 = nc.dram_tensor("iota32", [128, HI], F32, kind="ExternalInput").ap()
    return iota128, iota32


def _build_phase1(T: int):
    """Per-core partial computation: out 'partial' [128, 65] f16."""
    nc = bacc.Bacc("TRN2", target_bir_lowering=False, debug=False, num_devices=1)

    # blob_a: what the one-hot builds need first; blob_b: values
    # blob_a: [iota128(128) | iota32(32) | tlo,ulo,thi,uhi (4T)]
    # blob_b: [cbias(2) | tf,uf,p (3T)]
    AW = LO + HI + 4 * T
    BW = 2 + 3 * T
    bloba = nc.dram_tensor("bloba", [128, AW], F32, kind="ExternalInput").ap()
    blobb = nc.dram_tensor("blobb", [128, BW], F32, kind="ExternalInput").ap()
    partiald = nc.dram_tensor("partial", [128, 65], F16, kind="ExternalOutput").ap()

    with tile.TileContext(nc) as tc:
        with (
            tc.tile_pool(name="const", bufs=1) as cpool,
            tc.tile_pool(name="work", bufs=1) as wpool,
            tc.tile_pool(name="psum", bufs=1, space="PSUM") as ppool,
        ):
            # warm the Ln ACT table while the input DMA is in flight
            wz = cpool.tile([128, 1], F32, tag="wz")
            nc.vector.memset(wz[:], 0.5)
            wb = cpool.tile([128, 1], F32, tag="wb")
            nc.gpsimd.memset(wb[:], 0.0)
            wo = cpool.tile([128, 1], F32, tag="wo")
            nc.scalar.activation(wo[:], wz[:], ACT.Ln, bias=wb[:])

            ba = cpool.tile([128, AW], F32, tag="ba")
            nc.sync.dma_start(ba[:], bloba)
            bb = cpool.tile([128, BW], F32, tag="bb")
            nc.sync.dma_start(bb[:], blobb)

            io128 = ba[:, 0:LO]
            io32 = ba[:, LO:LO + HI]
            lo_pair = ba[:, LO + HI:LO + HI + 2 * T]
            hi_pair = ba[:, LO + HI + 2 * T:LO + HI + 4 * T]
            cb = bb[:, 0:2]
            tf = bb[:, 2:2 + T]
            uf = bb[:, 2 + T:2 + 2 * T]
            pp = bb[:, 2 + 2 * T:2 + 3 * T]

            C = wpool.tile([128, 65], F16, tag="C")

            # ---- one-hots (f16, exact), few wide DVE ops via stride-0 APs
            H_all = wpool.tile([128, 2 * T * HI], F16, tag="H_all")
            nc.vector.tensor_tensor(
                H_all[:].rearrange("p (t h) -> p t h", h=HI),
                io32.rearrange("p (o h) -> p o h", o=1).to_broadcast((128, 2 * T, HI)),
                hi_pair.rearrange("p (t o) -> p t o", o=1).to_broadcast((128, 2 * T, HI)),
                op=ALU.is_equal,
            )
            A_all = wpool.tile([128, 2 * T * LO], F16, tag="A_all")
            nc.vector.tensor_tensor(
                A_all[:].rearrange("p (t l) -> p t l", l=LO),
                io128.rearrange("p (o l) -> p o l", o=1).to_broadcast((128, 2 * T, LO)),
                lo_pair.rearrange("p (t o) -> p t o", o=1).to_broadcast((128, 2 * T, LO)),
                op=ALU.is_equal,
            )
            # V = [logmsg | p] on the ACT engine (parallel to the DVE ops)
            V = wpool.tile([128, 2 * T], F32, tag="V")
            nc.scalar.activation(V[:, 0:T], pp, ACT.Ln, scale=-1.0, bias=cb[:, 1:2])
            nc.scalar.copy(V[:, T:2 * T], pp)
            # small per-edge prep on GpSimd (parallel to the DVE ops)
            m = wpool.tile([128, T], F32, tag="m")
            nc.vector.tensor_tensor(m[:], tf, uf, op=ALU.is_equal)
            valu = wpool.tile([128, T], F32, tag="valu")   # p * (1 - m)
            nc.vector.scalar_tensor_tensor(
                valu[:], m[:], 0.5, pp, op0=ALU.is_lt, op1=ALU.mult
            )
            # dp2 = p^2 (2 - m) = (valu + p) * p, row-summed
            tsum = wpool.tile([128, T], F32, tag="tsum")
            nc.vector.tensor_tensor(tsum[:], valu[:], pp, op=ALU.add)
            dp2scr = wpool.tile([128, T], F32, tag="dp2scr")
            dp2r = wpool.tile([128, 1], F32, tag="dp2r")
            nc.vector.scalar_tensor_tensor(
                dp2scr[:], tsum[:], 1.0, pp,
                op0=ALU.mult, op1=ALU.mult, accum_out=dp2r[:],
            )

            # RS_all: per tile i the contiguous [rp_i(32) | rst_i(32)]
            RS_all = wpool.tile([128, T * 64], F16, tag="RS_all")
            nc.vector.tensor_tensor(
                RS_all[:].rearrange("p (t o h) -> p o t h", o=2, h=HI),
                H_all[:, 0:T * HI].rearrange("p (o t h) -> p o t h", o=1, h=HI)
                    .to_broadcast((128, 2, T, HI)),
                V[:].rearrange("p (o t) -> p o t", o=2)
                    .rearrange("p o (t h) -> p o t h", h=1)
                    .to_broadcast((128, 2, T, HI)),
                op=ALU.mult,
            )
            rsu_all = wpool.tile([128, T * HI], F16, tag="rsu_all")
            nc.vector.tensor_tensor(
                rsu_all[:].rearrange("p (t h) -> p t h", h=HI),
                H_all[:, T * HI:2 * T * HI].rearrange("p (t h) -> p t h", h=HI),
                valu[:].rearrange("p (t o) -> p t o", o=1).to_broadcast((128, T, HI)),
                op=ALU.mult,
            )

            # ---- scatter-add matmuls: P12 = [log_score(32) | s(32)]
            P12 = ppool.tile([128, 64], F32, tag="P12")
            for i in range(T):
                nc.tensor.matmul(
                    P12[:, 0:64],
                    A_all[:, i * LO:(i + 1) * LO],
                    RS_all[:, i * 64:(i + 1) * 64],
                    start=(i == 0), stop=False, skip_group_check=True,
                )
            for i in range(T):
                nc.tensor.matmul(
                    P12[:, 32:64],
                    A_all[:, (T + i) * LO:(T + i + 1) * LO],
                    rsu_all[:, i * HI:(i + 1) * HI],
                    start=False, stop=(i == T - 1), skip_group_check=True,
                )

            nc.scalar.copy(C[:, 0:64], P12[:])
            nc.scalar.copy(C[:, 64:65], dp2r[:])
            nc.sync.dma_start(partiald, C[:])

    nc.compile()
    return nc


def _build_phase2():
    """Combine 8 partials -> final scalar. Runs on one core."""
    nc = bacc.Bacc("TRN2", target_bir_lowering=False, debug=False, num_devices=1)

    partsd = nc.dram_tensor("parts", [128, 8 * 65], F16, kind="ExternalInput").ap()
    # misc: [ones(1) | cbias(2) | btail(64; row 0 holds batch[-64:], batch is
    #        sorted by construction so max(batch) = max(btail))]
    miscd = nc.dram_tensor("misc", [128, 67], F32, kind="ExternalInput").ap()
    outd = nc.dram_tensor("out", [1, 1], F32, kind="ExternalOutput").ap()

    with tile.TileContext(nc) as tc:
        with (
            tc.tile_pool(name="pool", bufs=1) as pool,
            tc.tile_pool(name="psum", bufs=1, space="PSUM") as ppool,
        ):
            wz = pool.tile([128, 1], F32, tag="wz")
            nc.vector.memset(wz[:], 0.5)
            wb = pool.tile([128, 1], F32, tag="wb")
            nc.gpsimd.memset(wb[:], 0.0)
            wo = pool.tile([128, 1], F32, tag="wo")
            nc.scalar.activation(wo[:], wz[:], ACT.Exp, bias=wb[:])

            mi = pool.tile([128, 67], F32, tag="mi")
            nc.sync.dma_start(mi[:], miscd)
            pt = pool.tile([128, 8 * 65], F16, tag="pt")
            nc.sync.dma_start(pt[:], partsd)
            ones_t = mi[:, 0:1]
            cb = mi[:, 1:3]
            bzero = cb[:, 0:1]

            # num_graphs = max(batch)+1 via the sorted tail, partition 0 only
            ng = pool.tile([1, 1], F32, tag="ng")
            nc.vector.tensor_reduce(ng[:], mi[0:1, 3:67], axis=AX.X, op=ALU.max)
            ng1 = pool.tile([1, 1], F32, tag="ng1")
            nc.vector.tensor_scalar_add(ng1[:], ng[:], 1.0)
            rng = pool.tile([1, 1], F32, tag="rng")
            nc.vector.reciprocal(rng[:], ng1[:])

            # 8-way partial sum in one strided reduce: C2[p,x] = sum_c pt[p,c*65+x]
            C2 = pool.tile([128, 65], F32, tag="C2")
            nc.vector.tensor_reduce(
                C2[:], pt[:].rearrange("p (c x) -> p x c", c=8), axis=AX.X, op=ALU.add
            )

            R = pool.tile([128, 3], F32, tag="R")
            scr1 = pool.tile([128, HI], F32, tag="scr1")
            nc.scalar.activation(scr1[:], C2[:, 0:32], ACT.Exp, bias=bzero,
                                 accum_out=R[:, 0:1])
            scr2 = pool.tile([128, HI], F32, tag="scr2")
            nc.vector.scalar_tensor_tensor(
                scr2[:], C2[:, 32:64], 1.0, C2[:, 32:64],
                op0=ALU.mult, op1=ALU.mult, accum_out=R[:, 1:2],
            )
            nc.vector.tensor_copy(R[:, 2:3], C2[:, 64:65])

            F = ppool.tile([1, 3], F32, tag="F")
            nc.tensor.matmul(F[:], ones_t, R[:], start=True, stop=True)
            Fs = pool.tile([1, 3], F32, tag="Fs")
            nc.scalar.copy(Fs[:], F[:])

            l2 = pool.tile([1, 1], F32, tag="l2")
            nc.vector.tensor_scalar(
                l2[:], Fs[:, 0:1], -float(PAD_NODES), PENALTY_SCALE / N_NODES,
                op0=ALU.add, op1=ALU.mult,
            )
            d32 = pool.tile([1, 1], F32, tag="d32")
            nc.vector.tensor_tensor(d32[:], Fs[:, 1:2], Fs[:, 2:3], op=ALU.subtract)
            t2s = pool.tile([1, 1], F32, tag="t2s")
            nc.vector.scalar_tensor_tensor(
                t2s[:], d32[:], 100.0, rng[:], op0=ALU.mult, op1=ALU.mult
            )
            res = pool.tile([1, 1], F32, tag="res")
            nc.vector.tensor_tensor(res[:], l2[:], t2s[:], op=ALU.add)
            nc.sync.dma_start(outd, res[:])

    nc.compile()
    return nc


def _pack_core(tt, uu, p, T):
    """Pack one core's edge shard into the [128, 7*T] fp32 edata layout."""
    ne = tt.shape[0]
    npad = T * 128

    def pad(a, fill):
        out = np.full(npad, fill, np.float64)
        out[:ne] = a
        return out.reshape(T, 128).T.astype(np.float32)  # [128, T]

    t_lo = pad(tt % 128, 0.0)
    t_hi = pad(tt // 128, float(HI))     # sentinel hi -> matches nothing
    u_lo = pad(uu % 128, 0.0)
    u_hi = pad(uu // 128, float(HI))
    tf = pad(tt, 0.0)
    uf = pad(uu, 0.0)                    # pad: tf==uf -> m=1, but p=0
    pf = pad(p, 0.0)
    return np.concatenate([t_lo, u_lo, t_hi, u_hi, tf, uf, pf], axis=1)


_CACHE = {}


def _get(name, builder, *a):
    if name not in _CACHE:
        _CACHE[name] = builder(*a)
    return _CACHE[name]


def kernel(x, edge_index, edge_feature, batch, _trace=False):
    x = np.asarray(x)
    ei = np.asarray(edge_index).astype(np.int64)
    p = np.asarray(edge_feature).astype(np.float32)[:, 0]
    batch = np.asarray(batch).astype(np.int64)

    uu_all = ei[0].astype(np.float64)
    tt_all = ei[1].astype(np.float64)

    iota128 = np.tile(np.arange(LO, dtype=np.float32), (128, 1))
    iota32 = np.tile(np.arange(HI, dtype=np.float32), (128, 1))
    ones = np.ones((128, 1), np.float32)
    cbias = np.zeros((128, 2), np.float32)
    cbias[:, 1] = 1.0 + 1e-6

    # ---- phase 1: per-core partials (no cross-core dependencies)
    nc1 = _get("p1", _build_phase1, TPC)
    in_maps = []
    for c in range(N_CORES):
        sl = slice(c * EPC, (c + 1) * EPC)
        ed = _pack_core(tt_all[sl], uu_all[sl], p[sl], TPC)
        T = TPC
        bloba = np.concatenate([iota128, iota32, ed[:, 0:4 * T]], axis=1)
        blobb = np.concatenate([cbias, ed[:, 4 * T:7 * T]], axis=1)
        in_maps.append({"bloba": bloba, "blobb": blobb})
    r1 = bass_utils.run_bass_kernel_spmd(
        nc1, in_maps, core_ids=list(range(N_CORES)), trace=_trace
    )

    # gather/unshard the per-core partials (pure data movement)
    parts = np.concatenate(
        [np.asarray(r1.results[c]["partial"]) for c in range(N_CORES)], axis=1
    ).astype(np.float16)

    # ---- phase 2: combine on one core
    nc2 = _get("p2", _build_phase2)
    btail = np.zeros((128, 64), np.float32)
    btail[0, :] = batch[-64:].astype(np.float32)
    misc = np.concatenate([ones, cbias, btail], axis=1)
    r2 = bass_utils.run_bass_kernel_spmd(
        nc2, [{"parts": parts, "misc": misc}], core_ids=[0], trace=_trace,
    )
    out = np.asarray(r2.results[0]["out"], dtype=np.float32).reshape(1, 1)
    if _trace:
        kernel.last_results = (r1, r2)
    return out
